# revision 27
# baseline (speedup 1.0000x reference)
"""DeepSeekMoE Trainium2 kernel — expert-parallel over 8 NeuronCores.

Strategy (self-contained; shapes hardcoded for the graded problem):
  - Each core owns 4 routed experts (expert-parallel). Router weights are
    column-PERMUTED per core so its 4 experts are always logits columns 0..3
    -> identical SPMD program on every core (no partition-id needed).
  - Router in exact fp32 (top-6 selection must match the fp32 reference
    ordering; measured rank6/7 logit gaps go down to 1.3e-5).
  - Top-6 via DVE max8 + match_replace on raw logits; gates =
    exp(l-max)*mask / sum  (softmax denominator cancels under top-k renorm).
  - Dispatch: counting-sort positions via triangular-matrix matmuls (prefix
    sums), then slot->token inverse maps via is_equal indicators + matmuls
    (static capacity CAP per expert; pad slots -> token 0 with gate == 0).
  - Gather token rows by indirect DMA, PE-transpose to d-major, expert MLP
    in float32r (full-speed PE), transpose back, gate-scale, indirect
    scatter-ADD into a per-core partial output. Shared experts run on a 1/8
    token slice, output d-major (host transposes).
  - Host: permute/tile weights per core, run SPMD on 8 cores, sum partials.
"""

import os
from contextlib import ExitStack
from dataclasses import dataclass

import numpy as np

import concourse.bass as bass
import concourse.tile as tile
from concourse import bacc, mybir
from concourse.bass_utils import run_bass_kernel_spmd
from concourse.masks import make_identity, make_upper_triangular

P = 128
F32 = mybir.dt.float32
F32R = mybir.dt.float32r
I32 = mybir.dt.int32
AX = mybir.AxisListType
ALU = mybir.AluOpType
ACT = mybir.ActivationFunctionType
BIGCHUNK = 512


@dataclass(frozen=True)
class Cfg:
    T: int = 4096          # total tokens
    D: int = 2048          # model dim
    H: int = 1408          # hidden dim
    E: int = 32            # routed experts (global)
    EPC: int = 4           # routed experts per core
    NSH: int = 2           # shared experts
    TOPK: int = 6
    CAP: int = 896         # per-expert token capacity (slots)
    NCORES: int = 8

    @property
    def KK(self):
        return self.D // P

    @property
    def HT(self):
        return self.H // P

    @property
    def TT(self):
        return self.T // P

    @property
    def TSH(self):
        return self.T // self.NCORES

    @property
    def ST(self):
        return self.CAP // P

    @property
    def CAPC(self):
        return len(self.CHUNKS)

    @property
    def CHUNKS(self):
        out = [BIGCHUNK] * (self.CAP // BIGCHUNK)
        if self.CAP % BIGCHUNK:
            out.append(self.CAP % BIGCHUNK)
        return out

    @property
    def DT(self):
        return self.D // P


CFG = Cfg()


def build_program(cfg: Cfg, fake_scatter: bool = False):
    """Build the SPMD Bass program (identical on every core)."""
    nc = bacc.Bacc("TRN2", target_bir_lowering=False, debug=False,
                   num_devices=cfg.NCORES)

    D, H, T, E, EPC, NSH = cfg.D, cfg.H, cfg.T, cfg.E, cfg.EPC, cfg.NSH
    KK, HT, TT, TSH, ST, CAPC, DT, CAP = (cfg.KK, cfg.HT, cfg.TT, cfg.TSH,
                                          cfg.ST, cfg.CAPC, cfg.DT, cfg.CAP)
    RC = max(1, T // BIGCHUNK)
    RCW = T // RC

    d = {}

    def din(name, shape, dt):
        d[name] = nc.dram_tensor(name, shape, dt, kind="ExternalInput").ap()

    def dout(name, shape, dt):
        d[name] = nc.dram_tensor(name, shape, dt, kind="ExternalOutput").ap()

    din("xT", [D, T], F32)
    din("xr", [T + 1, D], F32R)
    din("xts", [D, TSH], F32R)
    din("w1t", [EPC, HT, P, KK, P], F32R)
    din("w2t", [EPC, HT, P, D], F32R)
    din("b1t", [P, EPC * HT], F32)
    din("b2r", [EPC, D], F32R)
    din("sw1t", [NSH, HT, P, KK, P], F32R)
    din("sw2t", [NSH, HT, P, D], F32R)
    din("sb1t", [P, NSH * HT], F32)
    din("sb2s", [1, D], F32R)
    din("rw", [D, E], F32)
    din("rb", [E, 1], F32)
    din("lgrp", [P, P], F32)
    din("tokid", [P, TT], F32R)
    din("iota1p", [P, CAP], F32)
    din("onesr", [1, BIGCHUNK], F32R)
    dout("partial", [T + 1, D], F32)
    dout("outsh", [D, TSH], F32)

    with ExitStack() as octx:
        tc = octx.enter_context(tile.TileContext(nc))

        # -------- persistent consts (live through expert phase) --------
        pers = octx.enter_context(tc.tile_pool(name="pers", bufs=1))
        routing_ctx = octx.enter_context(ExitStack())
        rstate = routing_ctx.enter_context(tc.tile_pool(name="rstate", bufs=1))
        identf = rstate.tile([P, P], F32)
        make_identity(nc, identf[:])
        identr = pers.tile([P, P], F32R)
        nc.vector.tensor_copy(identr[:], identf[:])
        lstrict = rstate.tile([P, P], F32)
        make_upper_triangular(nc, lstrict[:], val=1.0, diag=False)
        onescol = rstate.tile([P, 1], F32)
        nc.vector.memset(onescol[:], 1.0)
        ones1r = rstate.tile([1, P], F32)
        nc.vector.memset(ones1r[:], 1.0)
        lgrp = rstate.tile([P, P], F32)
        nc.sync.dma_start(lgrp[:], d["lgrp"][:])
        tokid = rstate.tile([P, TT], F32R)
        nc.sync.dma_start(tokid[:], d["tokid"][:])
        iota1p = rstate.tile([P, CAP], F32)
        nc.sync.dma_start(iota1p[:], d["iota1p"][:])
        onesr = pers.tile([1, BIGCHUNK], F32R)
        nc.sync.dma_start(onesr[:], d["onesr"][:])
        tconstF = rstate.tile([1, 1], F32)
        nc.vector.memset(tconstF[:], float(T))
        tconstR = rstate.tile([1, 1], F32R)
        nc.vector.tensor_copy(tconstR[:], tconstF[:])

        gatesAll = rstate.tile([P, P], F32)    # col tau*EPC+j
        nc.vector.memset(gatesAll[:], 0.0)
        gatesAllR = rstate.tile([P, P], F32R)  # f32r copy for inverse matmuls
        # (no memset: f32r memset fails ISA check; phase T writes every read col)
        maskAll = rstate.tile([P, P], F32)
        nc.vector.memset(maskAll[:], 0.0)
        posm1 = rstate.tile([P, P], F32)
        idxAll = pers.tile([P, EPC * ST], I32)   # slot -> token id
        gSlot = pers.tile([P, EPC * ST], F32)    # slot -> gate

        # ================= Phase R + T: router, top-6, gates ===========
        with ExitStack() as rctx:
            rsb = rctx.enter_context(tc.tile_pool(name="router_sb", bufs=3))
            rps = rctx.enter_context(tc.tile_pool(name="router_ps", bufs=2, space="PSUM"))
            lsb = rctx.enter_context(tc.tile_pool(name="logits_sb", bufs=1))
            tsb = rctx.enter_context(tc.tile_pool(name="top6_sb", bufs=3))
            tps = rctx.enter_context(tc.tile_pool(name="top6_ps", bufs=2, space="PSUM"))

            rwt = rsb.tile([P, KK, E], F32, tag="rwt")
            nc.sync.dma_start(rwt[:], d["rw"].rearrange("(kk p) e -> p kk e", p=P))
            rbt = rsb.tile([E, 1], F32, tag="rbt")
            nc.sync.dma_start(rbt[:], d["rb"][:])
            logits32 = lsb.tile([E, T], F32)

            for rc in range(RC):
                xtc = rsb.tile([P, KK, RCW], F32, tag="xtc")
                nc.sync.dma_start(
                    xtc[:],
                    d["xT"].rearrange("(kk p) t -> p kk t", p=P)[:, :, rc * RCW:(rc + 1) * RCW])
                pr = rps.tile([E, RCW], F32, tag="pr")
                for kk in range(KK):
                    nc.tensor.matmul(pr[:], rwt[:, kk], xtc[:, kk],
                                     start=(kk == 0), stop=(kk == KK - 1))
                nc.vector.tensor_scalar_add(logits32[:, rc * RCW:(rc + 1) * RCW],
                                            pr[:], rbt[:, :1])

            for tau in range(TT):
                plg = tps.tile([P, E], F32, tag="plg")
                nc.tensor.transpose(plg[:], logits32[:, tau * P:(tau + 1) * P], identf[:E, :E])
                lg = tsb.tile([P, E], F32, tag="lg")
                nc.any.tensor_copy(lg[:], plg[:])
                m8 = tsb.tile([P, 8], F32, tag="m8")
                nc.vector.max(m8[:], lg[:])
                if cfg.TOPK < 8:
                    nc.vector.memset(m8[:, cfg.TOPK:8], -1e30)
                rest = tsb.tile([P, E], F32, tag="rest")
                nc.vector.match_replace(rest[:], in_to_replace=m8[:],
                                        in_values=lg[:], imm_value=-1e30)
                msk = tsb.tile([P, E], F32, tag="msk")
                nc.vector.tensor_scalar(msk[:], rest[:], -1e30, None, op0=ALU.is_equal)
                mx = tsb.tile([P, 1], F32, tag="mx")
                nc.vector.reduce_max(mx[:], lg[:], axis=AX.X)
                nmx = tsb.tile([P, 1], F32, tag="nmx")
                nc.vector.tensor_scalar_mul(nmx[:], mx[:], -1.0)
                ex = tsb.tile([P, E], F32, tag="ex")
                nc.scalar.activation(ex[:], lg[:], ACT.Exp, bias=nmx[:, :1], scale=1.0)
                exm = tsb.tile([P, E], F32, tag="exm")
                nc.vector.tensor_mul(exm[:], ex[:], msk[:])
                s6 = tsb.tile([P, 1], F32, tag="s6")
                nc.vector.reduce_sum(s6[:], exm[:], axis=AX.X)
                r6 = tsb.tile([P, 1], F32, tag="r6")
                nc.vector.reciprocal(r6[:], s6[:])
                gsl = gatesAll[:, tau * EPC:(tau + 1) * EPC]
                nc.vector.tensor_scalar_mul(gsl, exm[:, :EPC], r6[:, :1])
                nc.vector.tensor_copy(gatesAllR[:, tau * EPC:(tau + 1) * EPC], gsl)
                nc.vector.tensor_scalar(maskAll[:, tau * EPC:(tau + 1) * EPC],
                                        gsl, 0.0, None, op0=ALU.is_gt)

        # ================= Phase P: counting-sort positions ============
        with ExitStack() as pctx:
            psb = pctx.enter_context(tc.tile_pool(name="pos_sb", bufs=2))
            pps = pctx.enter_context(tc.tile_pool(name="pos_ps", bufs=2, space="PSUM"))
            ppsP = pctx.enter_context(tc.tile_pool(name="posP_ps", bufs=1, space="PSUM"))

            psumP = ppsP.tile([P, P], F32, tag="psumP")
            nc.tensor.matmul(psumP[:], lstrict[:], maskAll[:], start=True, stop=False)
            psumT = pps.tile([1, P], F32, tag="scr")
            nc.tensor.matmul(psumT[:], onescol[:], maskAll[:], start=True, stop=True)
            trow = psb.tile([1, P], F32, tag="trow")
            nc.any.tensor_copy(trow[:], psumT[:])
            ptc = pps.tile([P, 1], F32, tag="scr")
            nc.tensor.transpose(ptc[:], trow[:], identf[:1, :1])
            tcol = psb.tile([P, 1], F32, tag="tcol")
            nc.any.tensor_copy(tcol[:], ptc[:])
            po = pps.tile([P, 1], F32, tag="scr")
            nc.tensor.matmul(po[:], lgrp[:], tcol[:], start=True, stop=True)
            ocol = psb.tile([P, 1], F32, tag="ocol")
            nc.any.tensor_copy(ocol[:], po[:])
            por = pps.tile([1, P], F32, tag="scr")
            nc.tensor.transpose(por[:], ocol[:], identf[:])  # [128,1] in, full identity
            orow = psb.tile([1, P], F32, tag="orow")
            nc.any.tensor_copy(orow[:], por[:])
            nc.tensor.matmul(psumP[:], ones1r[:], orow[:], start=False, stop=True)
            # posm1 = (pos + 1) * mask   (0 where unselected; 1-based slots)
            nc.vector.scalar_tensor_tensor(posm1[:], psumP[:], 1.0, maskAll[:],
                                           op0=ALU.add, op1=ALU.mult)

        # ================= Phase I: slot -> (token, gate) maps =========
        with ExitStack() as ictx:
            isb = ictx.enter_context(tc.tile_pool(name="inv_sb", bufs=3))
            ips = ictx.enter_context(tc.tile_pool(name="inv_ps", bufs=2, space="PSUM"))
            ipt = ictx.enter_context(tc.tile_pool(name="invt_ps", bufs=2, space="PSUM"))
            for e in range(EPC):
                cbase = 0
                for c, CW in enumerate(cfg.CHUNKS):
                    pI1 = ips.tile([1, BIGCHUNK], F32, tag="pI1")
                    pI2 = ips.tile([1, BIGCHUNK], F32, tag="pI2")
                    # +T bias: pads (no indicator hit) resolve to trash row T
                    nc.tensor.matmul(pI1[:, :CW], tconstR[:], onesr[:, :CW],
                                     start=True, stop=False)
                    for tau in range(TT):
                        col = tau * EPC + e
                        ind = isb.tile([P, BIGCHUNK], F32R, tag="ind")
                        nc.vector.tensor_tensor(
                            ind[:, :CW],
                            posm1[:, col:col + 1].to_broadcast([P, CW]),
                            iota1p[:, cbase:cbase + CW],
                            op=ALU.is_equal)
                        nc.tensor.matmul(pI1[:, :CW], tokid[:, tau:tau + 1], ind[:, :CW],
                                         start=False, stop=(tau == TT - 1))
                        nc.tensor.matmul(pI2[:, :CW], gatesAllR[:, col:col + 1], ind[:, :CW],
                                         start=(tau == 0), stop=(tau == TT - 1))
                    irow = isb.tile([1, BIGCHUNK], F32, tag="irow")
                    nc.any.tensor_copy(irow[:, :CW], pI1[:, :CW])
                    grow = isb.tile([1, BIGCHUNK], F32, tag="grow")
                    nc.any.tensor_copy(grow[:, :CW], pI2[:, :CW])
                    for s4 in range(CW // P):
                        scol = e * ST + cbase // P + s4
                        pt1 = ipt.tile([P, 1], F32, tag="ptx")
                        nc.tensor.transpose(pt1[:], irow[:, s4 * P:(s4 + 1) * P], identf[:1, :1])
                        nc.any.tensor_copy(idxAll[:, scol:scol + 1], pt1[:])
                        pt2 = ipt.tile([P, 1], F32, tag="ptx")
                        nc.tensor.transpose(pt2[:], grow[:, s4 * P:(s4 + 1) * P], identf[:1, :1])
                        nc.any.tensor_copy(gSlot[:, scol:scol + 1], pt2[:])
                    cbase += CW

        routing_ctx.close()

        # ================= Phase S: shared experts =====================
        shared_ctx = octx.enter_context(ExitStack())
        if True:
            ssb = shared_ctx.enter_context(tc.tile_pool(name="sh_sb", bufs=2))
            h1p = shared_ctx.enter_context(tc.tile_pool(name="sh_h1", bufs=1))
            sps1 = shared_ctx.enter_context(tc.tile_pool(name="sh_ps1", bufs=2, space="PSUM"))
            sps2 = shared_ctx.enter_context(tc.tile_pool(name="sh_ps2", bufs=1, space="PSUM"))
            sb1 = ssb.tile([P, NSH * HT], F32, tag="sb1")
            nc.sync.dma_start(sb1[:], d["sb1t"][:])
            sb2 = ssb.tile([1, D], F32R, tag="sb2")
            nc.sync.dma_start(sb2[:], d["sb2s"][:])
            xsh = h1p.tile([P, KK, TSH], F32R)
            nc.sync.dma_start(xsh[:], d["xts"].rearrange("(kk p) t -> p kk t", p=P))
            h1sh = h1p.tile([P, NSH * HT, TSH], F32R)

            for es in range(NSH):
                for ht in range(HT):
                    w1 = ssb.tile([P, KK, P], F32R, tag="sw1")
                    nc.sync.dma_start(w1[:], d["sw1t"][es, ht])
                    ps1 = sps1.tile([P, TSH], F32, tag="ps1")
                    for kk in range(KK):
                        nc.tensor.matmul(ps1[:], w1[:, kk], xsh[:, kk],
                                         start=(kk == 0), stop=(kk == KK - 1))
                    nc.scalar.activation(h1sh[:, es * HT + ht], ps1[:], ACT.Relu,
                                         bias=sb1[:, es * HT + ht:es * HT + ht + 1],
                                         scale=1.0)

            for dtg in range(0, DT, 4):
                ndt = min(4, DT - dtg)
                psums = [sps2.tile([P, TSH], F32, tag=f"ps2_{i}", name=f"shps2_{i}") for i in range(ndt)]
                for es in range(NSH):
                    for hk in range(HT):
                        w2 = ssb.tile([P, 4 * P], F32R, tag="sw2")
                        nc.sync.dma_start(w2[:, :ndt * P],
                                          d["sw2t"][es, hk][:, dtg * P:(dtg + ndt) * P])
                        first = (es == 0 and hk == 0)
                        for i in range(ndt):
                            nc.tensor.matmul(psums[i][:], w2[:, i * P:(i + 1) * P],
                                             h1sh[:, es * HT + hk],
                                             start=first, stop=False)
                for i in range(ndt):
                    nc.tensor.matmul(psums[i][:],
                                     sb2[:, (dtg + i) * P:(dtg + i + 1) * P],
                                     onesr[:, :TSH], start=False, stop=True)
                    o = ssb.tile([P, TSH], F32, tag="sho")
                    nc.scalar.activation(o[:], psums[i][:], ACT.Copy, scale=0.5)
                    nc.sync.dma_start(d["outsh"][(dtg + i) * P:(dtg + i + 1) * P, :], o[:])

        shared_ctx.close()

        # ================= Phase E: routed experts =====================
        with ExitStack() as ectx:
            esb = ectx.enter_context(tc.tile_pool(name="ex_sb", bufs=2))
            w1p = ectx.enter_context(tc.tile_pool(name="ex_w1", bufs=2))
            b1p = ectx.enter_context(tc.tile_pool(name="ex_b1", bufs=1))
            xtp = ectx.enter_context(tc.tile_pool(name="ex_xtg", bufs=1))
            h1pool = ectx.enter_context(tc.tile_pool(name="ex_h1", bufs=1))
            ysb = ectx.enter_context(tc.tile_pool(name="ex_y", bufs=ST))
            eps1 = ectx.enter_context(tc.tile_pool(name="ex_ps1", bufs=1, space="PSUM"))
            eps2 = ectx.enter_context(tc.tile_pool(name="ex_ps2", bufs=1, space="PSUM"))
            epst = ectx.enter_context(tc.tile_pool(name="ex_pst", bufs=1, space="PSUM"))

            b1sb = b1p.tile([P, EPC * HT], F32, tag="b1sb")
            nc.sync.dma_start(b1sb[:], d["b1t"][:])
            NCH = len(cfg.CHUNKS)
            # dt groups sized so groups*NCH <= 6 psum banks
            gsz = max(1, 6 // NCH)
            dt_groups = []
            dtp = 0
            while dtp < DT:
                g = min(gsz, DT - dtp)
                dt_groups.append((dtp, g))
                dtp += g

            def emit_scatter(scol, yt, e, s):
                if fake_scatter:
                    row = (scol * P) % T
                    nc.gpsimd.dma_start(d["partial"][row:row + P, :], yt[:])
                else:
                    nc.gpsimd.indirect_dma_start(
                        out=d["partial"][:],
                        out_offset=bass.IndirectOffsetOnAxis(
                            ap=idxAll[:, scol:scol + 1], axis=0),
                        in_=yt[:], in_offset=None,
                        compute_op=ALU.add)

            pending_scatters = []

            def flush_scatters():
                for fn in pending_scatters:
                    fn()
                pending_scatters.clear()

            for e in range(EPC):
                b2 = b1p.tile([1, D], F32R, tag="b2")
                nc.sync.dma_start(b2[:], d["b2r"][e:e + 1, :])
                h1s = []
                for c, CW in enumerate(cfg.CHUNKS):
                    cbase = sum(cfg.CHUNKS[:c])
                    xtg = xtp.tile([P, KK, CW], F32R, tag=f"xtg_{c}",
                                   name=f"xtg_{e}_{c}")
                    for st in range(CW // P):
                        scol = e * ST + cbase // P + st
                        xg = esb.tile([P, D], F32R, tag="xg")
                        nc.gpsimd.indirect_dma_start(
                            out=xg[:], out_offset=None,
                            in_=d["xr"][:],
                            in_offset=bass.IndirectOffsetOnAxis(
                                ap=idxAll[:, scol:scol + 1], axis=0))
                        for kkg in range(0, KK, 4):
                            nb = min(4, KK - kkg)
                            pX = epst.tile([P, 4 * P], F32R, tag="pT")
                            for j in range(nb):
                                nc.tensor.transpose(
                                    pX[:, j * P:(j + 1) * P],
                                    xg[:, (kkg + j) * P:(kkg + j + 1) * P],
                                    identr[:])
                            nc.any.tensor_copy(
                                xtg[:, kkg:kkg + nb, st * P:(st + 1) * P],
                                pX[:, :nb * P].rearrange("p (b c) -> p b c", b=nb))

                    h1 = h1pool.tile([P, HT, CW], F32R, tag=f"h1_{c}",
                                     name=f"h1_{e}_{c}")
                    for ht in range(HT):
                        w1 = w1p.tile([P, KK, P], F32R, tag="w1")
                        nc.sync.dma_start(w1[:], d["w1t"][e, ht])
                        ps1 = eps1.tile([P, BIGCHUNK], F32, tag="ps1")
                        for kk in range(KK):
                            nc.tensor.matmul(ps1[:, :CW], w1[:, kk], xtg[:, kk],
                                             start=(kk == 0), stop=(kk == KK - 1))
                        nc.scalar.activation(h1[:, ht], ps1[:, :CW], ACT.Relu,
                                             bias=b1sb[:, e * HT + ht:e * HT + ht + 1],
                                             scale=1.0)
                    h1s.append(h1)
                # joint matmul2 over all chunks (w2 read once per expert)
                ytiles = [ysb.tile([P, D], F32, tag="y", name=f"y_{e}_{i}")
                          for i in range(ST)]
                for dtg, ndt in dt_groups:
                    psums = [eps2.tile([P, BIGCHUNK], F32, tag=f"p2_{i}",
                                       name=f"ep2_{i}")
                             for i in range(ndt * NCH)]
                    for hk in range(HT):
                        w2 = esb.tile([P, gsz * P], F32R, tag="w2")
                        nc.sync.dma_start(w2[:, :ndt * P],
                                          d["w2t"][e, hk][:, dtg * P:(dtg + ndt) * P])
                        for c, CW in enumerate(cfg.CHUNKS):
                            for i in range(ndt):
                                nc.tensor.matmul(
                                    psums[c * ndt + i][:, :CW],
                                    w2[:, i * P:(i + 1) * P],
                                    h1s[c][:, hk], start=(hk == 0), stop=False)
                    for c, CW in enumerate(cfg.CHUNKS):
                        cbase = sum(cfg.CHUNKS[:c])
                        for i in range(ndt):
                            dt = dtg + i
                            ps = psums[c * ndt + i]
                            nc.tensor.matmul(ps[:, :CW], b2[:, dt * P:(dt + 1) * P],
                                             onesr[:, :CW], start=False, stop=True)
                            stg = esb.tile([P, BIGCHUNK], F32R, tag="stg")
                            nc.any.tensor_copy(stg[:, :CW], ps[:, :CW])
                            nst = CW // P
                            pY = epst.tile([P, 4 * P], F32R, tag="pT")
                            for st in range(nst):
                                nc.tensor.transpose(pY[:, st * P:(st + 1) * P],
                                                    stg[:, st * P:(st + 1) * P],
                                                    identr[:])
                            for st in range(nst):
                                scol = e * ST + cbase // P + st
                                nc.vector.tensor_scalar_mul(
                                    ytiles[cbase // P + st][:, dt * P:(dt + 1) * P],
                                    pY[:, st * P:(st + 1) * P],
                                    gSlot[:, scol:scol + 1])
                for s in range(ST):
                    scol = e * ST + s
                    if True:
                        emit_scatter(scol, ytiles[s], e, s)
                        continue
                    if fake_scatter:
                        # timing-only variant: cost model charges indirect
                        # scatter by the full out-AP; use a plain write of
                        # identical real shape instead (WRONG results)
                        row = (scol * P) % T
                        nc.gpsimd.dma_start(
                            d["partial"][row:row + P, :], ytiles[s][:])
                    else:
                        nc.gpsimd.indirect_dma_start(
                            out=d["partial"][:],
                            out_offset=bass.IndirectOffsetOnAxis(
                                ap=idxAll[:, scol:scol + 1], axis=0),
                            in_=ytiles[s][:], in_offset=None,
                            compute_op=ALU.add)
            flush_scatters()

    nc.compile()
    return nc


def host_prepare(inputs, cfg: Cfg):
    """Build per-core in_maps from the full (unsharded) inputs."""
    T, D, H, E, EPC = cfg.T, cfg.D, cfg.H, cfg.E, cfg.EPC
    KK, HT, TSH, CAP, TT = cfg.KK, cfg.HT, cfg.TSH, cfg.CAP, cfg.TT

    x = np.ascontiguousarray(np.asarray(inputs["x"]).reshape(T, D), dtype=np.float32)
    xT = np.ascontiguousarray(x.T)
    xpad = np.ascontiguousarray(np.vstack([x, np.zeros((1, D), np.float32)]))
    rw1 = np.asarray(inputs["rw1"], dtype=np.float32)
    rb1 = np.asarray(inputs["rb1"], dtype=np.float32)
    rw2 = np.asarray(inputs["rw2"], dtype=np.float32)
    rb2 = np.asarray(inputs["rb2"], dtype=np.float32)
    sw1 = np.asarray(inputs["sw1"], dtype=np.float32)
    sb1 = np.asarray(inputs["sb1"], dtype=np.float32)
    sw2 = np.asarray(inputs["sw2"], dtype=np.float32)
    sb2 = np.asarray(inputs["sb2"], dtype=np.float32)
    router_w = np.asarray(inputs["router_w"], dtype=np.float32)
    router_b = np.asarray(inputs["router_b"], dtype=np.float32)

    def tile_w1(w):  # [n, D, H] -> [n, HT, P, KK, P]; per-partition 8KB lines
        n = w.shape[0]
        return np.ascontiguousarray(
            w.reshape(n, KK, P, HT, P).transpose(0, 3, 2, 1, 4))

    def tile_w2(w):  # [n, H, D] -> [n, HT, P, D]
        return np.ascontiguousarray(w.reshape(w.shape[0], HT, P, w.shape[2]))

    def tile_b1(b):  # [n, H] -> [P, n*HT]
        n = b.shape[0]
        return np.ascontiguousarray(
            b.reshape(n, HT, P).transpose(2, 0, 1).reshape(P, n * HT))

    sw1t, sw2t, sb1t = tile_w1(sw1), tile_w2(sw2), tile_b1(sb1)
    sb2s = sb2.sum(0, keepdims=True).astype(np.float32)

    lgrp = np.zeros((P, P), np.float32)
    pi = np.arange(P)
    lgrp[(pi[:, None] % EPC == pi[None, :] % EPC)
         & (pi[:, None] // EPC < pi[None, :] // EPC)] = 1.0
    tokid = (np.arange(TT)[None, :] * P + np.arange(P)[:, None] - T).astype(np.float32)
    tokid = np.ascontiguousarray(tokid)
    iota1p = np.ascontiguousarray(
        np.tile(np.arange(1, CAP + 1, dtype=np.float32)[None, :], (P, 1)))
    onesr = np.ones((1, BIGCHUNK), np.float32)

    in_maps = []
    for m in range(cfg.NCORES):
        mine = list(range(m * EPC, (m + 1) * EPC))
        rest = [e for e in range(E) if e not in mine]
        perm = mine + rest
        im = {
            "xT": xT,
            "xr": xpad,
            "xts": np.ascontiguousarray(xT[:, m * TSH:(m + 1) * TSH]),
            "w1t": tile_w1(rw1[mine]),
            "w2t": tile_w2(rw2[mine]),
            "b1t": tile_b1(rb1[mine]),
            "b2r": np.ascontiguousarray(rb2[mine]),
            "sw1t": sw1t, "sw2t": sw2t, "sb1t": sb1t, "sb2s": sb2s,
            "rw": np.ascontiguousarray(router_w[:, perm]),
            "rb": np.ascontiguousarray(router_b[perm]).reshape(E, 1),
            "lgrp": lgrp, "tokid": tokid, "iota1p": iota1p, "onesr": onesr,
        }
        in_maps.append(im)
    return in_maps


_PROG_CACHE = {}


def run_cores(inputs, cfg, trace=False):
    in_maps = host_prepare(inputs, cfg)
    if cfg not in _PROG_CACHE:
        _PROG_CACHE[cfg] = build_program(cfg)
    nc = _PROG_CACHE[cfg]
    return run_bass_kernel_spmd(nc, in_maps, core_ids=list(range(cfg.NCORES)),
                                trace=trace)


def combine(results, cfg, x_shape):
    out = np.zeros((cfg.T, cfg.D), np.float32)
    for m in range(cfg.NCORES):
        out += results[m]["partial"][:cfg.T]
        out[m * cfg.TSH:(m + 1) * cfg.TSH] += results[m]["outsh"].T
    return out.reshape(x_shape).astype(np.float32)


def kernel(**inputs) -> np.ndarray:
    cfg = CFG
    trace = bool(int(os.environ.get("MOE_TRACE", "0")))
    try:
        res = run_cores(inputs, cfg, trace=trace)
    except ModuleNotFoundError:
        res = run_cores(inputs, cfg, trace=False)
    if trace and res.exec_time_ns is not None:
        print(f"HW exec time: {res.exec_time_ns} ns")
    return combine(res.results, cfg, np.asarray(inputs["x"]).shape)
